# revision 1
# baseline (speedup 1.0000x reference)
"""LocallyConnected2d (3x3, stride 1) Trainium2 Bass kernel.

Shapes: x [64,32,64,64] f32, weight [1,64,32,62,62,9] f32 -> out [64,64,62,62] f32.

Strategy:
  - Shard output rows (OH=62, padded to 64) across 8 cores: 8 rows/core.
  - Per location (h,w): out[o,b] = sum_kj  Wc[96,o].T @ P[96,b], K-chunks of
    96 = (ki:3 x cin:32), accumulated in PSUM over kj=0..2.
  - Host pre-relayouts weight into the exact stationary-tile layout
    [h][p=(ki,i)][kj][w][o] (one perfectly-contiguous DMA per row) and ships
    x as per-h stacked 3-row tiles [p=(ki,i)][b][c]; the 3 kj-chunks are
    free-dim offsets (+0,+1,+2) of the same tile.
  - Output SBUF/PSUM tiles live at partitions 64-127 to balance SBUF AXI
    ports (even ports serve partitions 0-63, odd serve 64-127).
  - MODE:
      "fp32"   exact, but fp32 matmuls are 2-pass with serialized weight load
      "fp16"   both operands fp16 (rel err ~3e-4), ~2.7x faster PE [default]
      "bf16"   both operands bf16 (rel err ~2e-3), ~3.4x faster PE
      "bf16x3" split-fp32 compensation (rel err ~4e-6) — slower than fp32
               here because every matmul re-loads stationary weights
"""

import sys

if "/opt/trn_rl_repo" not in sys.path:
    sys.path.insert(0, "/opt/trn_rl_repo")

import numpy as np

B = 64
CIN = 32
H = W = 64
OH = OW = 62
COUT = 64
NCORES = 8
RH = 8  # padded output rows per core (8*8=64 >= 62)

MODE = "fp16"
TRACE = False
LAST = None

_PROGRAMS = {}


def _build_program(repeat=1, mode=None):
    mode = mode or MODE
    import concourse.bacc as bacc
    import concourse.mybir as mybir
    from concourse.tile import TileContext

    fp32 = mybir.dt.float32
    bf16 = mybir.dt.float16 if mode == "fp16" else mybir.dt.bfloat16
    nc = bacc.Bacc(
        "TRN2", target_bir_lowering=False, debug=False, num_devices=NCORES
    )

    wdt = fp32 if mode == "fp32" else bf16
    # input dram params; bf16x3 ships hi+lo planes stacked on a leading axis
    nw = 2 if mode == "bf16x3" else 1
    wt = nc.declare_dram_parameter("wt", [RH, 96, nw, 3, OW, COUT], wdt, isOutput=False)
    xs = nc.declare_dram_parameter("xs", [RH, 96, nw, B, W], wdt, isOutput=False)
    out = nc.declare_dram_parameter("out", [RH, COUT, OW, B], fp32, isOutput=True)

    # (wplane, xplane) matmul terms per (w, kj)
    if mode == "bf16x3":
        terms = [(0, 0), (1, 0), (0, 1)]  # wh.xh + wl.xh + wh.xl
    else:
        terms = [(0, 0)]

    GROUPS = [(g, min(8, OW - g)) for g in range(0, OW, 8)]

    with TileContext(nc) as tc:
        with (
            tc.tile_pool(name="wp", bufs=2) as wp,
            tc.tile_pool(name="xp", bufs=3) as xp,
            tc.tile_pool(name="op", bufs=2) as op,
            tc.tile_pool(name="pp", bufs=4, space="PSUM") as pp,
        ):
            for h in [hh for _ in range(repeat) for hh in range(RH)]:
                wtile = wp.tile([96, nw, 3, OW, COUT], wdt, tag="w")
                nc.sync.dma_start(out=wtile[:], in_=wt[h])
                xtile = xp.tile([96, nw, B, W], wdt, tag="x")
                nc.sync.dma_start(out=xtile[:], in_=xs[h])
                otile = op.tile([128, OW, B], fp32, tag="o")
                for w0, g in GROUPS:
                    ps = pp.tile([128, 8, B], fp32, tag="ps")
                    for wi in range(g):
                        w = w0 + wi
                        nmm = len(terms) * 3
                        mi = 0
                        for kj in range(3):
                            for (pw, px) in terms:
                                nc.tensor.matmul(
                                    ps[64:128, wi, :],
                                    lhsT=wtile[:, pw, kj, w, :],
                                    rhs=xtile[:, px, :, w + kj],
                                    start=(mi == 0),
                                    stop=(mi == nmm - 1),
                                )
                                mi += 1
                    nc.vector.tensor_copy(
                        otile[64:128, w0 : w0 + g, :], ps[64:128, 0:g, :]
                    )
                nc.sync.dma_start(out=out[h], in_=otile[64:128])
    nc.compile()
    return nc


def _split_bf16(a):
    import ml_dtypes

    hi = a.astype(ml_dtypes.bfloat16)
    lo = (a - hi.astype(np.float32)).astype(ml_dtypes.bfloat16)
    return hi, lo


def _prep_inputs(x, weight, mode=None):
    mode = mode or MODE
    import ml_dtypes

    x = np.ascontiguousarray(x, dtype=np.float32)
    weight = np.ascontiguousarray(weight, dtype=np.float32)

    xpad = np.zeros((B, CIN, H + 2, W), np.float32)
    xpad[:, :, :H, :] = x

    # weight -> [h, p=(ki*32+i), kj, w, o]
    w6 = weight[0].reshape(COUT, CIN, OH, OW, 3, 3)  # o,i,h,w,ki,kj
    wtr = np.transpose(w6, (2, 4, 1, 5, 3, 0))  # h,ki,i,kj,w,o
    wtpad = np.zeros((NCORES * RH, 3, CIN, 3, OW, COUT), np.float32)
    wtpad[:OH] = wtr
    wtpad = wtpad.reshape(NCORES * RH, 96, 3, OW, COUT)

    if mode == "fp32":
        wplanes = wtpad[:, :, None]  # [64,96,1,3,62,64] f32
    elif mode == "fp16":
        wplanes = wtpad.astype(np.float16)[:, :, None]
    elif mode == "bf16":
        wplanes = wtpad.astype(ml_dtypes.bfloat16)[:, :, None]
    else:
        wh, wl = _split_bf16(wtpad)
        wplanes = np.stack([wh, wl], axis=2)  # [64,96,2,3,62,64] bf16

    in_maps = []
    for c in range(NCORES):
        r0 = RH * c
        xw = xpad[:, :, r0 : r0 + RH + 2, :]  # [b,i,RH+2,c]
        sv = np.lib.stride_tricks.sliding_window_view(xw, 3, axis=2)  # b,i,RH,c,ki
        xs_c = np.ascontiguousarray(np.transpose(sv, (2, 4, 1, 0, 3)))  # h,ki,i,b,c
        xs_c = xs_c.reshape(RH, 96, B, W)
        if mode == "fp32":
            xplanes = xs_c[:, :, None]
        elif mode == "fp16":
            xplanes = xs_c.astype(np.float16)[:, :, None]
        elif mode == "bf16":
            xplanes = xs_c.astype(ml_dtypes.bfloat16)[:, :, None]
        else:
            xh, xl = _split_bf16(xs_c)
            xplanes = np.stack([xh, xl], axis=2)  # [RH,96,2,B,W]
        in_maps.append(
            {
                "wt": np.ascontiguousarray(wplanes[r0 : r0 + RH]),
                "xs": np.ascontiguousarray(xplanes),
            }
        )
    return in_maps


def kernel(x, weight):
    global LAST
    from concourse.bass_utils import run_bass_kernel_spmd

    if MODE not in _PROGRAMS:
        _PROGRAMS[MODE] = _build_program(mode=MODE)
    in_maps = _prep_inputs(np.asarray(x), np.asarray(weight))
    res = run_bass_kernel_spmd(
        _PROGRAMS[MODE], in_maps, list(range(NCORES)), trace=TRACE
    )
    LAST = res
    full = np.concatenate([r["out"] for r in res.results], axis=0)  # [64,o,w,b]
    return np.ascontiguousarray(np.transpose(full[:OH], (3, 1, 0, 2)))

